# revision 13
# baseline (speedup 1.0000x reference)
"""Trainium2 kernel for the conditional optimal diffusion score
(per-class masked-softmax RBF regression over the dataset).

Math (see reference): for query u, dataset x (N,D), labels y (N,):
    inner_n = -(0.5/sigma2) * ||u - s*x_n||^2,  s = sqrt(alpha_bar[t])
    w = per-class softmax of inner over {n : y_n == c}        (K,N)
    combo_c = sum_n w_nc x_n                                   (K,D)
    out = -(1/sigma2) * (u - s*combo)                          (K,D)

Strategy: shard x/y row-wise over 8 NeuronCores; each core makes one
streaming pass over its shard in fp16 (half the HBM bytes of fp32).
The per-sample logit uses the shift-square identity
    logit_n = c2 * ||x_n + h||^2 + const,   h = u/(2*rc), rc = c2/c1
so the per-tile work maps onto the engines as:
  DVE   : t = x + h   (fp16 tensor_tensor, 2x mode)
  ScalarE: q = rowsum(t^2)  (Square + accum, the critical path)
  ScalarE: e = exp(z) batched over groups of G tiles
  DVE   : z = c2*q - c2*Mq_y (one-hot gather of the per-class reference)
  PE    : V += wt^T x  (bf16 weights x fp16 moving),  S += wt^T 1
The per-class reference Mq_c (masked MIN of q) is estimated from the
first EST_T tiles; exactness is restored at merge time because the
reference cancels in V/S (flash-attention-style merge on host).
Weights are bf16 for range (z can reach ~+40 when the estimate
undershoots; e^z must not overflow a 16-bit float).

Padding: shards are padded from 6250 to 6272 rows (49*128) with
x_pad = 60000 (q ~ 1.1e13 -> z << 0 -> e = 0) and one-hot rows of
zeros (y_pad matches no class), so pad rows are doubly dead.
"""

import numpy as np

N, CH, HH, WW = 50000, 3, 32, 32
D = CH * HH * WW        # 3072
K = 10
NCORES = 8
NSHARD = N // NCORES    # 6250
P = 128
NT = 49                 # tiles per core
NPAD = NT * P           # 6272
FREE = 512              # matmul moving-operand slice
NSLICE = D // FREE      # 6
EST_T = 4               # tiles used for the per-class reference estimate
G = 5                   # tiles per exp group
BIG = 262144.0          # masked-min shift; q << BIG for randn-scale data
PAD_X = 60000.0

_NC_CACHE = {}
LAST_RESULTS = None


def _build_nc(c2: float, nt: int = NT):
    from contextlib import ExitStack

    import concourse.bacc as bacc
    import concourse.bass as bass
    import concourse.tile as tile
    from concourse import mybir

    f32 = mybir.dt.float32
    F16 = mybir.dt.float16
    BF = mybir.dt.bfloat16
    Alu = mybir.AluOpType
    Act = mybir.ActivationFunctionType

    nc = bacc.Bacc("TRN2", name="knn_softmax_score")

    npad = nt * P
    x_d = nc.dram_tensor("xs", [npad, D], F16, kind="ExternalInput")
    oh_d = nc.dram_tensor("ohs", [P, nt * K], BF, kind="ExternalInput")
    ohc_d = nc.dram_tensor("ohc", [P, K * nt], BF, kind="ExternalInput")
    h_d = nc.dram_tensor("hb", [D], F16, kind="ExternalInput")

    v_d = nc.dram_tensor("v_out", [K, D], f32, kind="ExternalOutput")
    s_d = nc.dram_tensor("s_out", [K, 1], f32, kind="ExternalOutput")
    g_d = nc.dram_tensor("g_out", [K, 1], f32, kind="ExternalOutput")
    est_dram = nc.dram_tensor("est_scratch", [EST_T * P, K], f32)
    mc_dram = nc.dram_tensor("mc_scratch", [K], f32)

    ngroups = (nt + G - 1) // G

    def grp(g):
        return range(g * G, min(nt, (g + 1) * G))

    with ExitStack() as ctx:
        tc = ctx.enter_context(tile.TileContext(nc))
        singles = ctx.enter_context(tc.tile_pool(name="singles", bufs=1))
        xpool = ctx.enter_context(tc.tile_pool(name="xpool", bufs=17))
        tpool = ctx.enter_context(tc.tile_pool(name="tpool", bufs=6))
        wpool = ctx.enter_context(tc.tile_pool(name="wpool", bufs=6))
        qepool = ctx.enter_context(tc.tile_pool(name="qepool", bufs=4))
        pspool = ctx.enter_context(tc.tile_pool(name="ps", bufs=1, space="PSUM"))

        # constants / broadcasts
        ub = singles.tile([P, D], F16, tag="ub")
        nc.gpsimd.dma_start(
            out=ub,
            in_=bass.AP(tensor=h_d[:].tensor, offset=0, ap=[[0, P], [1, D]]),
        )
        oh_all = singles.tile([P, nt * K], BF, tag="oh_all")
        nc.gpsimd.dma_start(out=oh_all, in_=oh_d[:, :])
        oh_cls = singles.tile([P, K * nt], BF, tag="oh_cls")
        nc.gpsimd.dma_start(out=oh_cls, in_=ohc_d[:, :])
        ones_col = singles.tile([P, 1], BF, tag="ones")
        nc.vector.memset(ones_col, 1.0)
        ones_f32 = singles.tile([P, 1], f32, tag="ones_f32")
        nc.vector.memset(ones_f32, 1.0)
        bmat = singles.tile([P, K], f32, tag="bmat")
        scr_b = singles.tile([P, nt], f32, tag="scr_b")

        act_scr = singles.tile([P, D], F16, tag="act_scr")
        q_all = singles.tile([P, nt], f32, tag="q_all")
        z_all = singles.tile([P, nt], f32, tag="z_all")
        e_all = singles.tile([P, nt], f32, tag="e_all")
        negm = singles.tile([P, K], f32, tag="negm")
        mc_col = singles.tile([K, 1], f32, tag="mc_col")
        est_rows = singles.tile([K, EST_T * P], f32, tag="est_rows")
        vsb = singles.tile([K, D], f32, tag="vsb")
        ssb = singles.tile([K, 1], f32, tag="ssb")

        psV = [
            pspool.tile([K, FREE], f32, tag=f"v{j}", name=f"psV{j}")
            for j in range(NSLICE)
        ]
        psS = pspool.tile([K, 1], f32, tag="s")
        psD = pspool.tile([1, 1], f32, tag="d")

        xts = {}

        def oh_t(t):
            return oh_all[:, t * K : (t + 1) * K]

        def head_a(t):
            """DMA + shift-add + square-accum for tile t."""
            xt = xpool.tile([P, D], F16, tag="xt", name=f"xt{t}")
            src = bass.AP(
                tensor=x_d[:].tensor, offset=t * P * D, ap=[[D, P], [1, D]]
            )
            nc.sync.dma_start(out=xt, in_=src)
            xts[t] = xt
            tt = tpool.tile([P, D], F16, tag="tt")
            nc.vector.tensor_tensor(tt, xt[:, :], ub[:, :], Alu.add)
            nc.scalar.activation(
                out=act_scr, in_=tt[:, :], func=Act.Square,
                accum_out=q_all[:, t : t + 1],
            )
            # 1-col keep-alive matmul gated on this tile's add: spreads PE
            # activity through the head phase so the HAM clock gate never
            # sees >3.4us idle and the real matmuls stay at 2.4 GHz.
            nc.tensor.matmul(
                psD, ones_col[:, :], tt[:, 0:1], start=True, stop=True
            )

        def head_b(t):
            """per-class reference gather + pre-activation z for tile t."""
            ohscr = qepool.tile([P, K], f32, tag="ohscr")
            biascol = qepool.tile([P, 1], f32, tag="bias")
            nc.vector.scalar_tensor_tensor(
                out=ohscr,
                in0=oh_t(t),
                scalar=1.0,
                op0=Alu.mult,
                in1=negm,
                op1=Alu.mult,
                accum_out=biascol,
            )
            nc.vector.tensor_scalar(
                z_all[:, t : t + 1], q_all[:, t : t + 1], c2, biascol[:, :],
                Alu.mult, Alu.add,
            )

        def pe_keepalive():
            """1-col matmul into a scratch bank so the HAM activity window
            never sees >3.4us of PE idle (cold PE runs at 1.2 instead of
            2.4 GHz, doubling every real matmul's duration)."""
            nc.tensor.matmul(
                psD, ones_col[:, :], ones_col[:, :], start=True, stop=True
            )

        def tail(t):
            """masked weights + PSUM matmul accumulation for tile t."""
            xt = xts.pop(t)
            wt = wpool.tile([P, K], BF, tag="wt")
            nc.vector.tensor_scalar(
                wt, oh_t(t), e_all[:, t : t + 1], None, Alu.mult
            )
            first, last = (t == 0), (t == nt - 1)
            for j in range(NSLICE):
                rhs = xt[:, j * FREE : (j + 1) * FREE]
                nc.tensor.matmul(psV[j], wt[:, :], rhs, start=first, stop=last)

        # --- estimate phase: per-class masked MIN of q over EST_T tiles ---
        for t in range(EST_T):
            head_a(t)
            shcol = qepool.tile([P, 1], f32, tag="sh")
            nc.vector.tensor_scalar(
                shcol, q_all[:, t : t + 1], -BIG, None, Alu.add
            )
            masked = qepool.tile([P, K], f32, tag="masked")
            nc.vector.tensor_scalar(
                masked, oh_t(t), shcol[:, :], BIG, Alu.mult, Alu.add
            )
            nc.gpsimd.dma_start(
                out=est_dram[t * P : (t + 1) * P, :], in_=masked
            )

        # heads for the next ~2 groups go out BEFORE the estimate chain so
        # the DVE/Act engines stay busy during the est DRAM round-trips
        next_head = EST_T
        while next_head < 3 * G:
            head_a(next_head)
            next_head += 1

        # per-class min over the EST_T*P estimate rows: read back transposed
        # (class-major), reduce along free dim, then round-trip through DRAM
        # to broadcast -c2*Mq_c to all partitions.
        nc.gpsimd.dma_start(
            out=est_rows,
            in_=bass.AP(
                tensor=est_dram[:].tensor, offset=0, ap=[[1, K], [K, EST_T * P]]
            ),
        )
        nc.vector.tensor_reduce(
            mc_col, est_rows, axis=mybir.AxisListType.X, op=Alu.min
        )
        nc.gpsimd.dma_start(out=mc_dram[:], in_=mc_col)
        nc.gpsimd.dma_start(
            out=negm,
            in_=bass.AP(tensor=mc_dram[:].tensor, offset=0, ap=[[0, P], [1, K]]),
        )
        nc.vector.tensor_scalar(negm, negm, -c2, None, Alu.mult)

        for t in range(3 * G):
            head_b(t)

        # --- software-pipelined main loop: heads run two exp-groups ahead,
        # and Exp(g) is emitted BEFORE the next head batch so the DVE adds
        # for upcoming tiles are never queued behind exp-gated weight ops ---
        for g in range(ngroups):
            ts_ = grp(g)
            a, b = ts_.start, ts_.stop
            nc.scalar.activation(
                out=e_all[:, a:b], in_=z_all[:, a:b], func=Act.Exp
            )
            target = min(nt, (g + 3) * G)
            while next_head < target:
                head_a(next_head)
                head_b(next_head)
                next_head += 1
            for t in ts_:
                tail(t)

        # S_c = sum_n oh_c * e via per-class masked reductions + one matmul
        # (replaces 49 per-tile psS matmuls on the PE critical path)
        for c in range(K):
            nc.vector.scalar_tensor_tensor(
                out=scr_b,
                in0=oh_cls[:, c * nt : (c + 1) * nt],
                scalar=1.0,
                op0=Alu.mult,
                in1=e_all[:, :],
                op1=Alu.mult,
                accum_out=bmat[:, c : c + 1],
            )
        nc.tensor.matmul(
            psS, bmat[:, :], ones_f32[:, :], start=True, stop=True
        )

        for j in range(NSLICE):
            dst = vsb[:, j * FREE : (j + 1) * FREE]
            if j % 2 == 0:
                nc.scalar.copy(out=dst, in_=psV[j][:, :])
            else:
                nc.vector.tensor_copy(dst, psV[j][:, :])
        nc.vector.tensor_copy(ssb, psS[:, :])
        nc.sync.dma_start(out=v_d[:, :], in_=vsb)
        nc.sync.dma_start(out=s_d[:, :], in_=ssb)
        nc.sync.dma_start(out=g_d[:, :], in_=mc_col)

    nc.finalize()
    return nc


def kernel(u, x_data, y, alpha_bar, t):
    from concourse.bass_utils import run_bass_kernel_spmd

    u = np.asarray(u, dtype=np.float32)
    x_data = np.asarray(x_data, dtype=np.float32)
    y = np.asarray(y)
    alpha_bar = np.asarray(alpha_bar, dtype=np.float32)
    ti = int(np.asarray(t))

    a_bar = float(alpha_bar[ti])
    s = float(np.sqrt(a_bar))
    sigma2 = 1.0 - a_bar
    c1 = s / sigma2
    c2 = -0.5 * s * s / sigma2
    rc = c2 / c1
    h = (u.reshape(-1) / (2.0 * rc)).astype(np.float16)

    key = np.float32(c2).item()
    if key not in _NC_CACHE:
        _NC_CACHE.clear()
        _NC_CACHE[key] = _build_nc(c2)
    nc = _NC_CACHE[key]

    x_flat = x_data.reshape(N, D)
    cls = np.arange(K, dtype=np.int64)

    in_maps = []
    for i in range(NCORES):
        xs = np.full((NPAD, D), PAD_X, dtype=np.float16)
        xs[:NSHARD] = x_flat[i * NSHARD : (i + 1) * NSHARD]
        ys = np.full((NPAD,), -1, dtype=np.int64)
        ys[:NSHARD] = y[i * NSHARD : (i + 1) * NSHARD]
        ohr = (ys[:, None] == cls[None, :]).astype(np.float32).reshape(NT, P, K)
        # [P, NT*K]: tile t's one-hot block contiguous (per-tile gather/mask)
        oh = np.ascontiguousarray(ohr.transpose(1, 0, 2).reshape(P, NT * K))
        # [P, K*NT]: class c's block contiguous (end-of-kernel S reduction)
        ohc = np.ascontiguousarray(ohr.transpose(1, 2, 0).reshape(P, K * NT))
        import ml_dtypes

        in_maps.append(
            {
                "xs": xs,
                "ohs": oh.astype(ml_dtypes.bfloat16),
                "ohc": ohc.astype(ml_dtypes.bfloat16),
                "hb": h,
            }
        )

    import os

    trace = os.environ.get("KNN_TRACE", "0") == "1"
    res = run_bass_kernel_spmd(
        nc, in_maps, core_ids=list(range(NCORES)), trace=trace
    )
    global LAST_RESULTS
    LAST_RESULTS = res

    # flash-attention style merge of the per-core softmax statistics
    Vs = np.stack([r["v_out"] for r in res.results]).astype(np.float64)
    Ss = np.stack([r["s_out"] for r in res.results]).astype(np.float64)
    Ms = np.stack([r["g_out"][:, 0] for r in res.results]).astype(np.float64)
    Ms = Ms * c2  # (ncores, K) logit-scale reference points
    f = np.exp(Ms - Ms.max(axis=0, keepdims=True))  # (ncores, K)
    V = np.einsum("ik,ikd->kd", f, Vs)
    S = np.einsum("ik,iko->ko", f, Ss)
    combo = V / S
    u_flat = u.reshape(1, D).astype(np.float64)
    result = -(1.0 / sigma2) * (u_flat - s * combo)
    return result.astype(np.float32).reshape(K, 1, CH, HH, WW)


# revision 15
# speedup vs baseline: 1.1743x; 1.1743x over previous
"""Trainium2 kernel for the conditional optimal diffusion score
(per-class masked-softmax RBF regression over the dataset).

Math (see reference): for query u, dataset x (N,D), labels y (N,):
    inner_n = -(0.5/sigma2) * ||u - s*x_n||^2,  s = sqrt(alpha_bar[t])
    w = per-class softmax of inner over {n : y_n == c}        (K,N)
    combo_c = sum_n w_nc x_n                                   (K,D)
    out = -(1/sigma2) * (u - s*combo)                          (K,D)

Strategy: shard x/y row-wise over 8 NeuronCores; each core makes one
streaming pass over its shard in fp16 (half the HBM bytes of fp32).
The per-sample logit uses the shift-square identity
    logit_n = c2 * ||x_n + h||^2 + const,   h = u/(2*rc), rc = c2/c1
so the per-tile work maps onto the engines as:
  DVE   : t = x + h   (fp16 tensor_tensor, 2x mode)
  ScalarE: q = rowsum(t^2)  (Square + accum, the critical path)
  ScalarE: e = exp(z) batched over groups of G tiles
  DVE   : z = c2*q - c2*Mq_y (one-hot gather of the per-class reference)
  PE    : V += wt^T x  (bf16 weights x fp16 moving),  S += wt^T 1
The per-class reference Mq_c (masked MIN of q) is estimated from the
first EST_T tiles; exactness is restored at merge time because the
reference cancels in V/S (flash-attention-style merge on host).
Weights are bf16 for range (z can reach ~+40 when the estimate
undershoots; e^z must not overflow a 16-bit float).

Padding: shards are padded from 6250 to 6272 rows (49*128) with
x_pad = 60000 (q ~ 1.1e13 -> z << 0 -> e = 0) and one-hot rows of
zeros (y_pad matches no class), so pad rows are doubly dead.
"""

import numpy as np

N, CH, HH, WW = 50000, 3, 32, 32
D = CH * HH * WW        # 3072
K = 10
NCORES = 8
NSHARD = N // NCORES    # 6250
P = 128
NT = 49                 # tiles per core
NPAD = NT * P           # 6272
FREE = 512              # matmul moving-operand slice
NSLICE = D // FREE      # 6
EST_T = 4               # tiles used for the per-class reference estimate
G = 5                   # tiles per exp group
BIG = 262144.0          # masked-min shift; q << BIG for randn-scale data
PAD_X = 60000.0

_NC_CACHE = {}
LAST_RESULTS = None


def _build_nc(c2: float, nt: int = NT):
    from contextlib import ExitStack

    import concourse.bacc as bacc
    import concourse.bass as bass
    import concourse.tile as tile
    from concourse import mybir

    f32 = mybir.dt.float32
    F16 = mybir.dt.float16
    BF = mybir.dt.bfloat16
    Alu = mybir.AluOpType
    Act = mybir.ActivationFunctionType

    nc = bacc.Bacc("TRN2", name="knn_softmax_score")

    npad = nt * P
    x_d = nc.dram_tensor("xs", [npad, D], F16, kind="ExternalInput")
    oh_d = nc.dram_tensor("ohs", [P, nt * K], BF, kind="ExternalInput")
    ohc_d = nc.dram_tensor("ohc", [P, K * nt], BF, kind="ExternalInput")
    h_d = nc.dram_tensor("hb", [D], F16, kind="ExternalInput")

    v_d = nc.dram_tensor("v_out", [K, D], f32, kind="ExternalOutput")
    s_d = nc.dram_tensor("s_out", [K, 1], f32, kind="ExternalOutput")
    g_d = nc.dram_tensor("g_out", [K, 1], f32, kind="ExternalOutput")
    est_dram = nc.dram_tensor("est_scratch", [EST_T * P, K], f32)
    mc_dram = nc.dram_tensor("mc_scratch", [K], f32)

    ngroups = (nt + G - 1) // G

    def grp(g):
        return range(g * G, min(nt, (g + 1) * G))

    with ExitStack() as ctx:
        tc = ctx.enter_context(tile.TileContext(nc))
        singles = ctx.enter_context(tc.tile_pool(name="singles", bufs=1))
        xpool = ctx.enter_context(tc.tile_pool(name="xpool", bufs=17))
        tpool = ctx.enter_context(tc.tile_pool(name="tpool", bufs=6))
        wpool = ctx.enter_context(tc.tile_pool(name="wpool", bufs=6))
        qepool = ctx.enter_context(tc.tile_pool(name="qepool", bufs=4))
        pspool = ctx.enter_context(tc.tile_pool(name="ps", bufs=1, space="PSUM"))

        # constants / broadcasts
        ub = singles.tile([P, D], F16, tag="ub")
        nc.gpsimd.dma_start(
            out=ub,
            in_=bass.AP(tensor=h_d[:].tensor, offset=0, ap=[[0, P], [1, D]]),
        )
        oh_all = singles.tile([P, nt * K], BF, tag="oh_all")
        nc.gpsimd.dma_start(out=oh_all, in_=oh_d[:, :])
        oh_cls = singles.tile([P, K * nt], BF, tag="oh_cls")
        nc.gpsimd.dma_start(out=oh_cls, in_=ohc_d[:, :])
        ones_col = singles.tile([P, 1], BF, tag="ones")
        nc.vector.memset(ones_col, 1.0)
        ones_f32 = singles.tile([P, 1], f32, tag="ones_f32")
        nc.vector.memset(ones_f32, 1.0)
        bmat = singles.tile([P, K], f32, tag="bmat")
        scr_b = singles.tile([P, nt], f32, tag="scr_b")

        act_scr = singles.tile([P, D], F16, tag="act_scr")
        q_all = singles.tile([P, nt], f32, tag="q_all")
        z_all = singles.tile([P, nt], f32, tag="z_all")
        e_all = singles.tile([P, nt], f32, tag="e_all")
        negm = singles.tile([P, K], f32, tag="negm")
        mc_col = singles.tile([K, 1], f32, tag="mc_col")
        est_rows = singles.tile([K, EST_T * P], f32, tag="est_rows")
        vsb = singles.tile([K, D], f32, tag="vsb")
        ssb = singles.tile([K, 1], f32, tag="ssb")

        psV = [
            pspool.tile([K, FREE], f32, tag=f"v{j}", name=f"psV{j}")
            for j in range(NSLICE)
        ]
        psS = pspool.tile([K, 1], f32, tag="s")
        psD = pspool.tile([1, 1], f32, tag="d")

        xts = {}

        def oh_t(t):
            return oh_all[:, t * K : (t + 1) * K]

        def head_a(t):
            """DMA + shift-add + square-accum for tile t."""
            xt = xpool.tile([P, D], F16, tag="xt", name=f"xt{t}")
            src = bass.AP(
                tensor=x_d[:].tensor, offset=t * P * D, ap=[[D, P], [1, D]]
            )
            nc.sync.dma_start(out=xt, in_=src)
            xts[t] = xt
            tt = tpool.tile([P, D], F16, tag="tt")
            nc.vector.tensor_tensor(tt, xt[:, :], ub[:, :], Alu.add)
            nc.scalar.activation(
                out=act_scr, in_=tt[:, :], func=Act.Square,
                accum_out=q_all[:, t : t + 1],
            )

        def keepalive(t):
            """1-col matmul gated on tile t's DMA: spreads PE activity
            through the head phase so the HAM clock gate never sees >3.4us
            idle (cold PE halves the clock of every real matmul). Emitted
            AFTER the current group's tails so it never queues ahead of a
            real matmul burst."""
            nc.tensor.matmul(
                psD, ones_col[:, :], xts[t][:, 0:1], start=True, stop=True
            )

        def head_b(t):
            """per-class reference gather + pre-activation z for tile t."""
            ohscr = qepool.tile([P, K], f32, tag="ohscr")
            biascol = qepool.tile([P, 1], f32, tag="bias")
            nc.vector.scalar_tensor_tensor(
                out=ohscr,
                in0=oh_t(t),
                scalar=1.0,
                op0=Alu.mult,
                in1=negm,
                op1=Alu.mult,
                accum_out=biascol,
            )
            nc.vector.tensor_scalar(
                z_all[:, t : t + 1], q_all[:, t : t + 1], c2, biascol[:, :],
                Alu.mult, Alu.add,
            )

        def pe_keepalive():
            """1-col matmul into a scratch bank so the HAM activity window
            never sees >3.4us of PE idle (cold PE runs at 1.2 instead of
            2.4 GHz, doubling every real matmul's duration)."""
            nc.tensor.matmul(
                psD, ones_col[:, :], ones_col[:, :], start=True, stop=True
            )

        def tail(t):
            """masked weights + PSUM matmul accumulation for tile t."""
            xt = xts.pop(t)
            wt = wpool.tile([P, K], BF, tag="wt")
            nc.vector.tensor_scalar(
                wt, oh_t(t), e_all[:, t : t + 1], None, Alu.mult
            )
            first, last = (t == 0), (t == nt - 1)
            for j in range(NSLICE):
                rhs = xt[:, j * FREE : (j + 1) * FREE]
                nc.tensor.matmul(psV[j], wt[:, :], rhs, start=first, stop=last)

        # --- estimate phase: per-class masked MIN of q over EST_T tiles ---
        for t in range(EST_T):
            head_a(t)
            shcol = qepool.tile([P, 1], f32, tag="sh")
            nc.vector.tensor_scalar(
                shcol, q_all[:, t : t + 1], -BIG, None, Alu.add
            )
            masked = qepool.tile([P, K], f32, tag="masked")
            nc.vector.tensor_scalar(
                masked, oh_t(t), shcol[:, :], BIG, Alu.mult, Alu.add
            )
            nc.gpsimd.dma_start(
                out=est_dram[t * P : (t + 1) * P, :], in_=masked
            )

        # heads for the next ~2 groups go out BEFORE the estimate chain so
        # the DVE/Act engines stay busy during the est DRAM round-trips
        next_head = EST_T
        while next_head < 3 * G:
            head_a(next_head)
            next_head += 1

        # per-class min over the EST_T*P estimate rows: read back transposed
        # (class-major), reduce along free dim, then round-trip through DRAM
        # to broadcast -c2*Mq_c to all partitions.
        nc.gpsimd.dma_start(
            out=est_rows,
            in_=bass.AP(
                tensor=est_dram[:].tensor, offset=0, ap=[[1, K], [K, EST_T * P]]
            ),
        )
        nc.vector.tensor_reduce(
            mc_col, est_rows, axis=mybir.AxisListType.X, op=Alu.min
        )
        nc.gpsimd.dma_start(out=mc_dram[:], in_=mc_col)
        nc.gpsimd.dma_start(
            out=negm,
            in_=bass.AP(tensor=mc_dram[:].tensor, offset=0, ap=[[0, P], [1, K]]),
        )
        nc.vector.tensor_scalar(negm, negm, -c2, None, Alu.mult)

        for t in range(3 * G):
            head_b(t)
        for t in range(2 * G):
            keepalive(t)

        # --- software-pipelined main loop: heads run two exp-groups ahead,
        # and Exp(g) is emitted BEFORE the next head batch so the DVE adds
        # for upcoming tiles are never queued behind exp-gated weight ops ---
        for g in range(ngroups):
            ts_ = grp(g)
            a, b = ts_.start, ts_.stop
            nc.scalar.activation(
                out=e_all[:, a:b], in_=z_all[:, a:b], func=Act.Exp
            )
            target = min(nt, (g + 3) * G)
            batch = range(next_head, target)
            while next_head < target:
                head_a(next_head)
                head_b(next_head)
                next_head += 1
            for t in ts_:
                tail(t)
            for t in batch:
                keepalive(t)

        # S_c = sum_n oh_c * e via per-class masked reductions + one matmul
        # (replaces 49 per-tile psS matmuls on the PE critical path)
        for c in range(K):
            nc.vector.scalar_tensor_tensor(
                out=scr_b,
                in0=oh_cls[:, c * nt : (c + 1) * nt],
                scalar=1.0,
                op0=Alu.mult,
                in1=e_all[:, :],
                op1=Alu.mult,
                accum_out=bmat[:, c : c + 1],
            )
        nc.tensor.matmul(
            psS, bmat[:, :], ones_f32[:, :], start=True, stop=True
        )

        for j in range(NSLICE):
            dst = vsb[:, j * FREE : (j + 1) * FREE]
            if j % 2 == 0:
                nc.scalar.copy(out=dst, in_=psV[j][:, :])
            else:
                nc.vector.tensor_copy(dst, psV[j][:, :])
        nc.vector.tensor_copy(ssb, psS[:, :])
        nc.sync.dma_start(out=v_d[:, :], in_=vsb)
        nc.sync.dma_start(out=s_d[:, :], in_=ssb)
        nc.sync.dma_start(out=g_d[:, :], in_=mc_col)

    nc.finalize()
    return nc


def kernel(u, x_data, y, alpha_bar, t):
    from concourse.bass_utils import run_bass_kernel_spmd

    u = np.asarray(u, dtype=np.float32)
    x_data = np.asarray(x_data, dtype=np.float32)
    y = np.asarray(y)
    alpha_bar = np.asarray(alpha_bar, dtype=np.float32)
    ti = int(np.asarray(t))

    a_bar = float(alpha_bar[ti])
    s = float(np.sqrt(a_bar))
    sigma2 = 1.0 - a_bar
    c1 = s / sigma2
    c2 = -0.5 * s * s / sigma2
    rc = c2 / c1
    h = (u.reshape(-1) / (2.0 * rc)).astype(np.float16)

    key = np.float32(c2).item()
    if key not in _NC_CACHE:
        _NC_CACHE.clear()
        _NC_CACHE[key] = _build_nc(c2)
    nc = _NC_CACHE[key]

    x_flat = x_data.reshape(N, D)
    cls = np.arange(K, dtype=np.int64)

    in_maps = []
    for i in range(NCORES):
        xs = np.full((NPAD, D), PAD_X, dtype=np.float16)
        xs[:NSHARD] = x_flat[i * NSHARD : (i + 1) * NSHARD]
        ys = np.full((NPAD,), -1, dtype=np.int64)
        ys[:NSHARD] = y[i * NSHARD : (i + 1) * NSHARD]
        ohr = (ys[:, None] == cls[None, :]).astype(np.float32).reshape(NT, P, K)
        # [P, NT*K]: tile t's one-hot block contiguous (per-tile gather/mask)
        oh = np.ascontiguousarray(ohr.transpose(1, 0, 2).reshape(P, NT * K))
        # [P, K*NT]: class c's block contiguous (end-of-kernel S reduction)
        ohc = np.ascontiguousarray(ohr.transpose(1, 2, 0).reshape(P, K * NT))
        import ml_dtypes

        in_maps.append(
            {
                "xs": xs,
                "ohs": oh.astype(ml_dtypes.bfloat16),
                "ohc": ohc.astype(ml_dtypes.bfloat16),
                "hb": h,
            }
        )

    import os

    trace = os.environ.get("KNN_TRACE", "0") == "1"
    res = run_bass_kernel_spmd(
        nc, in_maps, core_ids=list(range(NCORES)), trace=trace
    )
    global LAST_RESULTS
    LAST_RESULTS = res

    # flash-attention style merge of the per-core softmax statistics
    Vs = np.stack([r["v_out"] for r in res.results]).astype(np.float64)
    Ss = np.stack([r["s_out"] for r in res.results]).astype(np.float64)
    Ms = np.stack([r["g_out"][:, 0] for r in res.results]).astype(np.float64)
    Ms = Ms * c2  # (ncores, K) logit-scale reference points
    f = np.exp(Ms - Ms.max(axis=0, keepdims=True))  # (ncores, K)
    V = np.einsum("ik,ikd->kd", f, Vs)
    S = np.einsum("ik,iko->ko", f, Ss)
    combo = V / S
    u_flat = u.reshape(1, D).astype(np.float64)
    result = -(1.0 / sigma2) * (u_flat - s * combo)
    return result.astype(np.float32).reshape(K, 1, CH, HH, WW)
